# revision 10
# baseline (speedup 1.0000x reference)
"""Trainium2 Bass kernel for the MetalsLSTMBlock problem.

Computation (per batch row b):
    xn   = rms_norm(x[b]) * ln_weight                       # (S, D)
    pre_g = xn @ Wg.T                g in {i, f, o, c}      # (S, P)
    i,f,o = sigmoid(softcap(pre));  c = pre_c
    h_t  = f_t * h_{t-1} + i_t * tanh(c_t)   (scan over S)
    ot_t = o_t * tanh(h_t)
    out  = x[b] + ot @ Wout.T                               # (S, D)

Sharding: 8 cores = 4 batch rows x 2 halves of the projection dim P.
Each core computes all four gate projections for its P-half (padded to
1408 = 11*128 channels), runs the recurrence with channels on SBUF
partitions and time along the free dim (hardware tensor_tensor_scan),
then produces a partial output projection. Host sums the two partials
per batch (the j=1 core receives a zero residual) and concatenates the
hidden-state shards.

Matmuls run in bf16 (fp32 accumulation in PSUM); the recurrence runs in
fp32 on the vector engine.
"""

import numpy as np
from contextlib import ExitStack

B, S, D, P = 4, 2048, 2048, 2729
N_CORES = 8
P0 = 1365            # channels on j=0 cores; j=1 cores get P - P0 = 1364
NPT = 11             # channel tiles per core
PH = NPT * 128       # padded channels per core
CAP = 15.0
EPS = 1e-6

_NC = None


def build(SL=S, DL=D, npt=NPT):
    import concourse.bacc as bacc
    import concourse.tile as tile
    from concourse import mybir

    f32 = mybir.dt.float32
    bf16 = mybir.dt.bfloat16
    AF = mybir.ActivationFunctionType
    ALU = mybir.AluOpType

    KD = DL // 128        # contraction tiles for the gate matmuls
    NSC = SL // 512       # sequence chunks of 512
    ST = SL // 128        # sequence tiles of 128 (phase-2 M chunks)
    DT = (DL + 511) // 512
    ph = npt * 128

    nc = bacc.Bacc("TRN2", target_bir_lowering=False, debug=False,
                   num_devices=N_CORES)

    xt = nc.dram_tensor("xt", [DL, SL], bf16, kind="ExternalInput").ap()
    xr = nc.dram_tensor("xr", [SL, DL], f32, kind="ExternalInput").ap()
    wg = nc.dram_tensor("wg", [npt, 128, 4, KD, 128], bf16,
                        kind="ExternalInput").ap()
    wo = nc.dram_tensor("wo", [ph, DL], bf16, kind="ExternalInput").ap()
    h0 = nc.dram_tensor("h0", [ph], f32, kind="ExternalInput").ap()
    out = nc.dram_tensor("out", [SL, DL], f32, kind="ExternalOutput").ap()
    hf = nc.dram_tensor("hf", [ph], f32, kind="ExternalOutput").ap()

    with tile.TileContext(nc) as tc, ExitStack() as ctx:
        pxn = ctx.enter_context(tc.tile_pool(name="xn", bufs=1))
        pconst = ctx.enter_context(tc.tile_pool(name="const", bufs=1))
        ptmp = ctx.enter_context(tc.tile_pool(name="tmp", bufs=2))
        pwg = ctx.enter_context(tc.tile_pool(name="wgp", bufs=5))
        pot = ctx.enter_context(tc.tile_pool(name="otp", bufs=1))
        pw2 = ctx.enter_context(tc.tile_pool(name="wop", bufs=1))
        pio = ctx.enter_context(tc.tile_pool(name="iop", bufs=2))
        ppsum = ctx.enter_context(
            tc.tile_pool(name="psp", bufs=5, space="PSUM"))
        ppsum2 = ctx.enter_context(
            tc.tile_pool(name="psp2", bufs=3, space="PSUM"))

        # ---- phase 0: load x^T, compute rms-norm scale, normalize ----
        xn = pxn.tile([128, KD, SL], bf16, tag="xn")
        nc.sync.dma_start(xn[:], xt.rearrange("(k p) s -> p k s", p=128))

        ones = pconst.tile([128, 128], bf16, tag="ones")
        nc.any.memset(ones[:], 1.0)
        eps_t = pconst.tile([128, 1], f32, tag="eps")
        nc.any.memset(eps_t[:], EPS)
        h0s = pconst.tile([128, npt], f32, tag="h0s")
        nc.sync.dma_start(h0s[:], h0.rearrange("(t p) -> p t", p=128))

        # rb[p, s] = 1/sqrt(mean_d x[s,d]^2 + eps), identical on every
        # partition: the all-ones lhsT makes PE broadcast the column sums.
        rb = pconst.tile([128, SL], f32, tag="rb")
        for sc in range(NSC):
            ssl = slice(sc * 512, (sc + 1) * 512)
            ps = ppsum.tile([128, 512], f32, tag="ps")
            for kd in range(KD):
                x2 = ptmp.tile([128, 512], bf16, tag="x2")
                xc = xn[:, kd, ssl]
                nc.vector.tensor_mul(x2[:], xc, xc)
                nc.tensor.matmul(ps[:], ones[:], x2[:],
                                 start=(kd == 0), stop=(kd == KD - 1))
            sd = ptmp.tile([128, 512], f32, tag="sd")
            nc.scalar.activation(sd[:], ps[:], AF.Sqrt,
                                 bias=eps_t[:], scale=1.0 / DL)
            nc.vector.reciprocal(rb[:, ssl], sd[:])
        for kd in range(KD):
            nc.vector.tensor_mul(xn[:, kd, :], xn[:, kd, :], rb[:])

        # ---- phase 1: gates + recurrence, one 128-channel tile at a time
        ot = pot.tile([128, npt, SL], bf16, tag="ot")
        hfs = pconst.tile([128, npt], f32, tag="hfs")

        def gate_w(pt, gi):
            w = pwg.tile([128, KD, 128], bf16, tag="w")
            nc.sync.dma_start(w[:], wg[pt, :, gi])
            return w

        def gate_psum(w, ssl):
            ps = ppsum.tile([128, 512], f32, tag="ps")
            for kd in range(KD):
                nc.tensor.matmul(ps[:], w[:, kd, :], xn[:, kd, ssl],
                                 start=(kd == 0), stop=(kd == KD - 1))
            return ps

        for pt in range(npt):
            wqi = gate_w(pt, 0)
            wqc = gate_w(pt, 1)
            wqf = gate_w(pt, 2)
            wqo = gate_w(pt, 3)
            h_prev = None
            for sc in range(NSC):
                ssl = slice(sc * 512, (sc + 1) * 512)
                # i gate: sigmoid(softcap(.))
                ps_i = gate_psum(wqi, ssl)
                tt = ptmp.tile([128, 512], f32, tag="tt")
                nc.scalar.activation(tt[:], ps_i[:], AF.Tanh, scale=1.0 / CAP)
                sgi = ptmp.tile([128, 512], f32, tag="sgi")
                nc.scalar.activation(sgi[:], tt[:], AF.Sigmoid, scale=CAP)
                # c gate: tanh(.)
                ps_c = gate_psum(wqc, ssl)
                thc = ptmp.tile([128, 512], f32, tag="thc")
                nc.scalar.activation(thc[:], ps_c[:], AF.Tanh)
                u = ptmp.tile([128, 512], f32, tag="u")
                nc.vector.tensor_mul(u[:], sgi[:], thc[:])
                # f gate
                ps_f = gate_psum(wqf, ssl)
                tt2 = ptmp.tile([128, 512], f32, tag="tt")
                nc.scalar.activation(tt2[:], ps_f[:], AF.Tanh, scale=1.0 / CAP)
                sgf = ptmp.tile([128, 512], f32, tag="sgf")
                nc.scalar.activation(sgf[:], tt2[:], AF.Sigmoid, scale=CAP)
                # recurrence h = f*h + u along the free (time) axis
                hc = ptmp.tile([128, 512], f32, tag="h")
                init = h0s[:, pt:pt + 1] if sc == 0 else h_prev[:, 511:512]
                nc.vector.tensor_tensor_scan(hc[:], sgf[:], u[:], init,
                                             ALU.mult, ALU.add)
                h_prev = hc
                th = ptmp.tile([128, 512], f32, tag="th")
                nc.scalar.activation(th[:], hc[:], AF.Tanh)
                # o gate
                ps_o = gate_psum(wqo, ssl)
                tt3 = ptmp.tile([128, 512], f32, tag="tt")
                nc.scalar.activation(tt3[:], ps_o[:], AF.Tanh, scale=1.0 / CAP)
                sgo = ptmp.tile([128, 512], f32, tag="sgo")
                nc.scalar.activation(sgo[:], tt3[:], AF.Sigmoid, scale=CAP)
                nc.vector.tensor_mul(ot[:, pt, ssl], sgo[:], th[:])
                if sc == NSC - 1:
                    nc.vector.tensor_copy(hfs[:, pt:pt + 1], hc[:, 511:512])
        nc.sync.dma_start(hf.rearrange("(t p) -> p t", p=128), hfs[:])

        # ---- phase 2: output projection + residual ----
        wo_r = wo.rearrange("(t p) d -> p t d", p=128)
        for dt in range(DT):
            d0 = dt * 512
            dn = min(512, DL - d0)
            dsl = slice(d0, d0 + dn)
            wot = pw2.tile([128, npt, dn], bf16, tag="wot")
            nc.sync.dma_start(wot[:], wo_r[:, :, dsl])
            for st in range(ST):
                stsl = slice(st * 128, (st + 1) * 128)
                ps = ppsum2.tile([128, dn], f32, tag="ps2")
                for kt in range(npt):
                    nc.tensor.matmul(ps[:], ot[:, kt, stsl], wot[:, kt, :],
                                     start=(kt == 0), stop=(kt == npt - 1))
                res = pio.tile([128, dn], f32, tag="res")
                nc.sync.dma_start(res[:], xr[stsl, dsl])
                oc = pio.tile([128, dn], f32, tag="oc")
                nc.vector.tensor_add(oc[:], ps[:], res[:])
                nc.sync.dma_start(out[stsl, dsl], oc[:])

    nc.compile()
    return nc


def _shard_inputs(x, hidden_state, Wi, Wf, Wo, Wc, Wout, ln_weight):
    import ml_dtypes
    bf = ml_dtypes.bfloat16
    KD = D // 128

    # fold ln_weight into the gate weights; builder gate order is i,c,f,o
    gates = [Wi * ln_weight[None, :], Wc * ln_weight[None, :],
             Wf * ln_weight[None, :], Wo * ln_weight[None, :]]

    per_j = []
    for j in range(2):
        lo = 0 if j == 0 else P0
        hi = P0 if j == 0 else P
        n = hi - lo
        wgb = np.zeros((NPT, 128, 4, KD, 128), dtype=bf)
        for gi, Wg in enumerate(gates):
            pad = np.zeros((PH, D), dtype=np.float32)
            pad[:n] = Wg[lo:hi]
            r = pad.reshape(NPT, 128, KD, 128)          # [pt, m, kd, dp]
            wgb[:, :, gi, :, :] = r.transpose(0, 3, 2, 1).astype(bf)
        wob = np.zeros((PH, D), dtype=bf)
        wob[:n] = Wout[:, lo:hi].T.astype(bf)
        per_j.append((wgb, wob, lo, hi, n))

    zeros_res = np.zeros((S, D), dtype=np.float32)
    in_maps = []
    for b in range(B):
        xt_b = x[b].T.astype(bf)
        for j in range(2):
            wgb, wob, lo, hi, n = per_j[j]
            h0b = np.zeros(PH, dtype=np.float32)
            h0b[:n] = hidden_state[b, lo:hi]
            in_maps.append({
                "xt": xt_b,
                "xr": x[b] if j == 0 else zeros_res,
                "wg": wgb,
                "wo": wob,
                "h0": h0b,
            })
    return in_maps


def kernel(x, hidden_state, Wi, Wf, Wo, Wc, Wout, ln_weight, _trace=False):
    from concourse.bass_utils import run_bass_kernel_spmd

    x = np.asarray(x, dtype=np.float32)
    hidden_state = np.asarray(hidden_state, dtype=np.float32)
    Wi = np.asarray(Wi, dtype=np.float32)
    Wf = np.asarray(Wf, dtype=np.float32)
    Wo = np.asarray(Wo, dtype=np.float32)
    Wc = np.asarray(Wc, dtype=np.float32)
    Wout = np.asarray(Wout, dtype=np.float32)
    ln_weight = np.asarray(ln_weight, dtype=np.float32)

    global _NC
    if _NC is None:
        _NC = build()

    in_maps = _shard_inputs(x, hidden_state, Wi, Wf, Wo, Wc, Wout, ln_weight)
    res = run_bass_kernel_spmd(_NC, in_maps, list(range(N_CORES)),
                               trace=_trace)

    out = np.empty((B, S, D), dtype=np.float32)
    h_final = np.empty((B, P), dtype=np.float32)
    for b in range(B):
        r0, r1 = res.results[2 * b], res.results[2 * b + 1]
        out[b] = r0["out"] + r1["out"]
        h_final[b, :P0] = r0["hf"][:P0]
        h_final[b, P0:] = r1["hf"][:P - P0]
    if _trace:
        kernel.last_exec_time_ns = res.exec_time_ns
    return out, h_final


# revision 14
# speedup vs baseline: 1.0487x; 1.0487x over previous
"""Trainium2 Bass kernel for the MetalsLSTMBlock problem.

Computation (per batch row b):
    xn   = rms_norm(x[b]) * ln_weight                       # (S, D)
    pre_g = xn @ Wg.T                g in {i, f, o, c}      # (S, P)
    i,f,o = sigmoid(softcap(pre));  c = pre_c
    h_t  = f_t * h_{t-1} + i_t * tanh(c_t)   (scan over S)
    ot_t = o_t * tanh(h_t)
    out  = x[b] + ot @ Wout.T                               # (S, D)

Sharding: 8 cores = 4 batch rows x 2 halves of the projection dim P.
Each core computes all four gate projections for its P-half (padded to
1408 = 11*128 channels), runs the recurrence with channels on SBUF
partitions and time along the free dim (hardware tensor_tensor_scan),
then produces a partial output projection. Host sums the two partials
per batch (the j=1 core receives a zero residual) and concatenates the
hidden-state shards.

Matmuls run in bf16 (fp32 accumulation in PSUM); the recurrence runs in
fp32 on the vector engine.
"""

import numpy as np
from contextlib import ExitStack

B, S, D, P = 4, 2048, 2048, 2729
N_CORES = 8
P0 = 1365            # channels on j=0 cores; j=1 cores get P - P0 = 1364
NPT = 11             # channel tiles per core
PH = NPT * 128       # padded channels per core
CAP = 15.0
EPS = 1e-6

_NC = None


def build(SL=S, DL=D, npt=NPT):
    import concourse.bacc as bacc
    import concourse.tile as tile
    from concourse import mybir

    f32 = mybir.dt.float32
    bf16 = mybir.dt.bfloat16
    AF = mybir.ActivationFunctionType
    ALU = mybir.AluOpType

    KD = DL // 128        # contraction tiles for the gate matmuls
    NSC = SL // 512       # sequence chunks of 512
    ST = SL // 128        # sequence tiles of 128 (phase-2 M chunks)
    DT = (DL + 511) // 512
    ph = npt * 128

    nc = bacc.Bacc("TRN2", target_bir_lowering=False, debug=False,
                   num_devices=N_CORES)

    xt = nc.dram_tensor("xt", [DL, SL], bf16, kind="ExternalInput").ap()
    xr = nc.dram_tensor("xr", [SL, DL], f32, kind="ExternalInput").ap()
    wg = nc.dram_tensor("wg", [npt, 128, 4, KD, 128], bf16,
                        kind="ExternalInput").ap()
    wo = nc.dram_tensor("wo", [ph, DL], bf16, kind="ExternalInput").ap()
    h0 = nc.dram_tensor("h0", [ph], f32, kind="ExternalInput").ap()
    out = nc.dram_tensor("out", [SL, DL], f32, kind="ExternalOutput").ap()
    hf = nc.dram_tensor("hf", [ph], f32, kind="ExternalOutput").ap()

    with tile.TileContext(nc) as tc, ExitStack() as ctx:
        pxn = ctx.enter_context(tc.tile_pool(name="xn", bufs=1))
        pconst = ctx.enter_context(tc.tile_pool(name="const", bufs=1))
        ptmp = ctx.enter_context(tc.tile_pool(name="tmp", bufs=2))
        pwg = ctx.enter_context(tc.tile_pool(name="wgp", bufs=4))
        pot = ctx.enter_context(tc.tile_pool(name="otp", bufs=1))
        pw2 = ctx.enter_context(tc.tile_pool(name="wop", bufs=2))
        pio = ctx.enter_context(tc.tile_pool(name="iop", bufs=2))
        ppsum = ctx.enter_context(
            tc.tile_pool(name="psp", bufs=5, space="PSUM"))
        ppsum2 = ctx.enter_context(
            tc.tile_pool(name="psp2", bufs=3, space="PSUM"))

        # ---- phase 0: load x^T, compute rms-norm scale, normalize ----
        # One tile per 512-wide sequence chunk so later phases depend only
        # on the chunks they read (lets phase 1 start after chunk 0).
        xt_r = xt.rearrange("(k p) s -> p k s", p=128)
        xnc = []
        for sc in range(NSC):
            ssl = slice(sc * 512, (sc + 1) * 512)
            t = pxn.tile([128, KD, 512], bf16, tag=f"xn{sc}")
            nc.sync.dma_start(t[:], xt_r[:, :, ssl])
            xnc.append(t)

        ones = pconst.tile([128, 128], bf16, tag="ones")
        nc.any.memset(ones[:], 1.0)
        eps_t = pconst.tile([128, 1], f32, tag="eps")
        nc.any.memset(eps_t[:], EPS)
        h0s = pconst.tile([128, npt], f32, tag="h0s")
        nc.sync.dma_start(h0s[:], h0.rearrange("(t p) -> p t", p=128))

        # r[p, s] = 1/sqrt(mean_d x[s,d]^2 + eps), identical on every
        # partition: the all-ones lhsT makes PE broadcast the column sums.
        for sc in range(NSC):
            ps = ppsum.tile([128, 512], f32, tag="ps")
            for kd in range(KD):
                x2 = ptmp.tile([128, 512], bf16, tag="x2")
                xc = xnc[sc][:, kd, :]
                nc.scalar.activation(x2[:], xc, AF.Square)
                nc.tensor.matmul(ps[:], ones[:], x2[:],
                                 start=(kd == 0), stop=(kd == KD - 1))
            sd = ptmp.tile([128, 512], f32, tag="sd")
            nc.scalar.activation(sd[:], ps[:], AF.Sqrt,
                                 bias=eps_t[:], scale=1.0 / DL)
            rc = ptmp.tile([128, 512], f32, tag="rc")
            nc.vector.reciprocal_approx_fast(rc[:], sd[:])
            for kd in range(KD):
                nc.vector.tensor_mul(xnc[sc][:, kd, :], xnc[sc][:, kd, :],
                                     rc[:])

        # ---- phase 1: gates + recurrence, one 128-channel tile at a time
        ot = pot.tile([128, npt, SL], bf16, tag="ot")
        hfs = pconst.tile([128, npt], f32, tag="hfs")

        def gate_w(pt, gi):
            w = pwg.tile([128, KD, 128], bf16, tag="w")
            nc.sync.dma_start(w[:], wg[pt, :, gi])
            return w

        def gate_psum(w, sc):
            ps = ppsum.tile([128, 512], f32, tag="ps")
            for kd in range(KD):
                nc.tensor.matmul(ps[:], w[:, kd, :], xnc[sc][:, kd, :],
                                 start=(kd == 0), stop=(kd == KD - 1))
            return ps

        for pt in range(npt):
            wqi = gate_w(pt, 0)
            wqc = gate_w(pt, 1)
            wqf = gate_w(pt, 2)
            wqo = gate_w(pt, 3)
            h_prev = None
            for sc in range(NSC):
                ssl = slice(sc * 512, (sc + 1) * 512)
                # i gate: sigmoid(softcap(.))
                ps_i = gate_psum(wqi, sc)
                tt = ptmp.tile([128, 512], f32, tag="tt")
                nc.scalar.activation(tt[:], ps_i[:], AF.Tanh, scale=1.0 / CAP)
                sgi = ptmp.tile([128, 512], f32, tag="sgi")
                nc.scalar.activation(sgi[:], tt[:], AF.Sigmoid, scale=CAP)
                # c gate: tanh(.)
                ps_c = gate_psum(wqc, sc)
                thc = ptmp.tile([128, 512], f32, tag="thc")
                nc.scalar.activation(thc[:], ps_c[:], AF.Tanh)
                u = ptmp.tile([128, 512], f32, tag="u")
                nc.vector.tensor_mul(u[:], sgi[:], thc[:])
                # f gate
                ps_f = gate_psum(wqf, sc)
                tt2 = ptmp.tile([128, 512], f32, tag="tt")
                nc.scalar.activation(tt2[:], ps_f[:], AF.Tanh, scale=1.0 / CAP)
                sgf = ptmp.tile([128, 512], f32, tag="sgf")
                nc.scalar.activation(sgf[:], tt2[:], AF.Sigmoid, scale=CAP)
                # recurrence h = f*h + u along the free (time) axis
                hc = ptmp.tile([128, 512], f32, tag="h")
                init = h0s[:, pt:pt + 1] if sc == 0 else h_prev[:, 511:512]
                nc.vector.tensor_tensor_scan(hc[:], sgf[:], u[:], init,
                                             ALU.mult, ALU.add)
                h_prev = hc
                th = ptmp.tile([128, 512], f32, tag="th")
                nc.scalar.activation(th[:], hc[:], AF.Tanh)
                # o gate
                ps_o = gate_psum(wqo, sc)
                tt3 = ptmp.tile([128, 512], f32, tag="tt")
                nc.scalar.activation(tt3[:], ps_o[:], AF.Tanh, scale=1.0 / CAP)
                sgo = ptmp.tile([128, 512], f32, tag="sgo")
                nc.scalar.activation(sgo[:], tt3[:], AF.Sigmoid, scale=CAP)
                nc.vector.tensor_mul(ot[:, pt, ssl], sgo[:], th[:])
                if sc == NSC - 1:
                    nc.vector.tensor_copy(hfs[:, pt:pt + 1], hc[:, 511:512])
        nc.sync.dma_start(hf.rearrange("(t p) -> p t", p=128), hfs[:])

        # ---- phase 2: output projection + residual ----
        wo_r = wo.rearrange("(t p) d -> p t d", p=128)
        for dt in range(DT):
            d0 = dt * 512
            dn = min(512, DL - d0)
            dsl = slice(d0, d0 + dn)
            wot = pw2.tile([128, npt, dn], bf16, tag="wot")
            nc.sync.dma_start(wot[:], wo_r[:, :, dsl])
            for st in range(ST):
                stsl = slice(st * 128, (st + 1) * 128)
                ps = ppsum2.tile([128, dn], f32, tag="ps2")
                for kt in range(npt):
                    nc.tensor.matmul(ps[:], ot[:, kt, stsl], wot[:, kt, :],
                                     start=(kt == 0), stop=(kt == npt - 1))
                res = pio.tile([128, dn], f32, tag="res")
                nc.sync.dma_start(res[:], xr[stsl, dsl])
                oc = pio.tile([128, dn], f32, tag="oc")
                nc.vector.tensor_add(oc[:], ps[:], res[:])
                nc.sync.dma_start(out[stsl, dsl], oc[:])

    nc.compile()
    return nc


def _shard_inputs(x, hidden_state, Wi, Wf, Wo, Wc, Wout, ln_weight):
    import ml_dtypes
    bf = ml_dtypes.bfloat16
    KD = D // 128

    # fold ln_weight into the gate weights; builder gate order is i,c,f,o
    gates = [Wi * ln_weight[None, :], Wc * ln_weight[None, :],
             Wf * ln_weight[None, :], Wo * ln_weight[None, :]]

    per_j = []
    for j in range(2):
        lo = 0 if j == 0 else P0
        hi = P0 if j == 0 else P
        n = hi - lo
        wgb = np.zeros((NPT, 128, 4, KD, 128), dtype=bf)
        for gi, Wg in enumerate(gates):
            pad = np.zeros((PH, D), dtype=np.float32)
            pad[:n] = Wg[lo:hi]
            r = pad.reshape(NPT, 128, KD, 128)          # [pt, m, kd, dp]
            wgb[:, :, gi, :, :] = r.transpose(0, 3, 2, 1).astype(bf)
        wob = np.zeros((PH, D), dtype=bf)
        wob[:n] = Wout[:, lo:hi].T.astype(bf)
        per_j.append((wgb, wob, lo, hi, n))

    zeros_res = np.zeros((S, D), dtype=np.float32)
    in_maps = []
    for b in range(B):
        xt_b = x[b].T.astype(bf)
        for j in range(2):
            wgb, wob, lo, hi, n = per_j[j]
            h0b = np.zeros(PH, dtype=np.float32)
            h0b[:n] = hidden_state[b, lo:hi]
            in_maps.append({
                "xt": xt_b,
                "xr": x[b] if j == 0 else zeros_res,
                "wg": wgb,
                "wo": wob,
                "h0": h0b,
            })
    return in_maps


def kernel(x, hidden_state, Wi, Wf, Wo, Wc, Wout, ln_weight, _trace=False):
    from concourse.bass_utils import run_bass_kernel_spmd

    x = np.asarray(x, dtype=np.float32)
    hidden_state = np.asarray(hidden_state, dtype=np.float32)
    Wi = np.asarray(Wi, dtype=np.float32)
    Wf = np.asarray(Wf, dtype=np.float32)
    Wo = np.asarray(Wo, dtype=np.float32)
    Wc = np.asarray(Wc, dtype=np.float32)
    Wout = np.asarray(Wout, dtype=np.float32)
    ln_weight = np.asarray(ln_weight, dtype=np.float32)

    global _NC
    if _NC is None:
        _NC = build()

    in_maps = _shard_inputs(x, hidden_state, Wi, Wf, Wo, Wc, Wout, ln_weight)
    res = run_bass_kernel_spmd(_NC, in_maps, list(range(N_CORES)),
                               trace=_trace)

    out = np.empty((B, S, D), dtype=np.float32)
    h_final = np.empty((B, P), dtype=np.float32)
    for b in range(B):
        r0, r1 = res.results[2 * b], res.results[2 * b + 1]
        out[b] = r0["out"] + r1["out"]
        h_final[b, :P0] = r0["hf"][:P0]
        h_final[b, P0:] = r1["hf"][:P - P0]
    if _trace:
        kernel.last_exec_time_ns = res.exec_time_ns
    return out, h_final


# revision 17
# speedup vs baseline: 1.0537x; 1.0048x over previous
"""Trainium2 Bass kernel for the MetalsLSTMBlock problem.

Computation (per batch row b):
    xn   = rms_norm(x[b]) * ln_weight                       # (S, D)
    pre_g = xn @ Wg.T                g in {i, f, o, c}      # (S, P)
    i,f,o = sigmoid(softcap(pre));  c = pre_c
    h_t  = f_t * h_{t-1} + i_t * tanh(c_t)   (scan over S)
    ot_t = o_t * tanh(h_t)
    out  = x[b] + ot @ Wout.T                               # (S, D)

Sharding: 8 cores = 4 batch rows x 2 halves of the projection dim P.
Each core computes all four gate projections for its P-half (padded to
1408 = 11*128 channels), runs the recurrence with channels on SBUF
partitions and time along the free dim (hardware tensor_tensor_scan),
then produces a partial output projection. Host sums the two partials
per batch (the j=1 core receives a zero residual) and concatenates the
hidden-state shards.

Matmuls run in bf16 (fp32 accumulation in PSUM); the recurrence runs in
fp32 on the vector engine.
"""

import numpy as np
from contextlib import ExitStack

B, S, D, P = 4, 2048, 2048, 2729
N_CORES = 8
P0 = 1365            # channels on j=0 cores; j=1 cores get P - P0 = 1364
NPT = 11             # channel tiles per core
PH = NPT * 128       # padded channels per core
CAP = 15.0
EPS = 1e-6

_NC = None


def build(SL=S, DL=D, npt=NPT):
    import concourse.bacc as bacc
    import concourse.tile as tile
    from concourse import mybir

    f32 = mybir.dt.float32
    bf16 = mybir.dt.bfloat16
    AF = mybir.ActivationFunctionType
    ALU = mybir.AluOpType

    KD = DL // 128        # contraction tiles for the gate matmuls
    NSC = SL // 512       # sequence chunks of 512
    ST = SL // 128        # sequence tiles of 128 (phase-2 M chunks)
    DT = (DL + 511) // 512
    ph = npt * 128

    nc = bacc.Bacc("TRN2", target_bir_lowering=False, debug=False,
                   num_devices=N_CORES)

    xt = nc.dram_tensor("xt", [DL, SL], bf16, kind="ExternalInput").ap()
    xr = nc.dram_tensor("xr", [SL, DL], f32, kind="ExternalInput").ap()
    wg = nc.dram_tensor("wg", [npt, 128, 4, KD, 128], bf16,
                        kind="ExternalInput").ap()
    wo = nc.dram_tensor("wo", [ph, DL], bf16, kind="ExternalInput").ap()
    h0 = nc.dram_tensor("h0", [ph], f32, kind="ExternalInput").ap()
    out = nc.dram_tensor("out", [SL, DL], f32, kind="ExternalOutput").ap()
    hf = nc.dram_tensor("hf", [ph], f32, kind="ExternalOutput").ap()

    with tile.TileContext(nc) as tc, ExitStack() as ctx:
        pxn = ctx.enter_context(tc.tile_pool(name="xn", bufs=1))
        pconst = ctx.enter_context(tc.tile_pool(name="const", bufs=1))
        ptmp = ctx.enter_context(tc.tile_pool(name="tmp", bufs=2))
        pwg = ctx.enter_context(tc.tile_pool(name="wgp", bufs=4))
        pot = ctx.enter_context(tc.tile_pool(name="otp", bufs=1))
        pw2 = ctx.enter_context(tc.tile_pool(name="wop", bufs=2))
        pio = ctx.enter_context(tc.tile_pool(name="iop", bufs=2))
        ppsum = ctx.enter_context(
            tc.tile_pool(name="psp", bufs=4, space="PSUM"))
        ppsum2 = ctx.enter_context(
            tc.tile_pool(name="psp2", bufs=4, space="PSUM"))

        # ---- phase 0: load x^T, compute rms-norm scale, normalize ----
        # One tile per 512-wide sequence chunk so later phases depend only
        # on the chunks they read (lets phase 1 start after chunk 0).
        xt_r = xt.rearrange("(k p) s -> p k s", p=128)
        xnc = []
        for sc in range(NSC):
            ssl = slice(sc * 512, (sc + 1) * 512)
            t = pxn.tile([128, KD, 512], bf16, tag=f"xn{sc}")
            nc.sync.dma_start(t[:], xt_r[:, :, ssl])
            xnc.append(t)

        ones = pconst.tile([128, 128], bf16, tag="ones")
        nc.any.memset(ones[:], 1.0)
        eps_t = pconst.tile([128, 1], f32, tag="eps")
        nc.any.memset(eps_t[:], EPS)
        h0s = pconst.tile([128, npt], f32, tag="h0s")
        nc.sync.dma_start(h0s[:], h0.rearrange("(t p) -> p t", p=128))

        # r[p, s] = 1/sqrt(mean_d x[s,d]^2 + eps), identical on every
        # partition: the all-ones lhsT makes PE broadcast the column sums.
        for sc in range(NSC):
            ps = ppsum.tile([128, 512], f32, tag="ps")
            for kd in range(KD):
                x2 = ptmp.tile([128, 512], bf16, tag="x2")
                xc = xnc[sc][:, kd, :]
                if kd % 2 == 0:
                    nc.scalar.activation(x2[:], xc, AF.Square)
                else:
                    nc.vector.tensor_mul(x2[:], xc, xc)
                nc.tensor.matmul(ps[:], ones[:], x2[:],
                                 start=(kd == 0), stop=(kd == KD - 1))
            sd = ptmp.tile([128, 512], f32, tag="sd")
            nc.scalar.activation(sd[:], ps[:], AF.Sqrt,
                                 bias=eps_t[:], scale=1.0 / DL)
            rc = ptmp.tile([128, 512], f32, tag="rc")
            nc.vector.reciprocal_approx_fast(rc[:], sd[:])
            for kd in range(KD):
                nc.vector.tensor_mul(xnc[sc][:, kd, :], xnc[sc][:, kd, :],
                                     rc[:])

        # ---- phase 1: gates + recurrence, one 128-channel tile at a time
        ot = pot.tile([128, npt, SL], bf16, tag="ot")
        hfs = pconst.tile([128, npt], f32, tag="hfs")

        def gate_w(pt, gi):
            w = pwg.tile([128, KD, 128], bf16, tag="w")
            nc.sync.dma_start(w[:], wg[pt, :, gi])
            return w

        def gate_psum(w, sc):
            ps = ppsum.tile([128, 512], f32, tag="ps")
            for kd in range(KD):
                nc.tensor.matmul(ps[:], w[:, kd, :], xnc[sc][:, kd, :],
                                 start=(kd == 0), stop=(kd == KD - 1))
            return ps

        for pt in range(npt):
            wqi = gate_w(pt, 0)
            wqc = gate_w(pt, 1)
            wqf = gate_w(pt, 2)
            wqo = gate_w(pt, 3)
            h_prev = None
            for sc in range(NSC):
                ssl = slice(sc * 512, (sc + 1) * 512)
                # i gate: sigmoid(softcap(.))
                ps_i = gate_psum(wqi, sc)
                tt = ptmp.tile([128, 512], f32, tag="tt")
                nc.scalar.activation(tt[:], ps_i[:], AF.Tanh, scale=1.0 / CAP)
                sgi = ptmp.tile([128, 512], f32, tag="sgi")
                nc.scalar.activation(sgi[:], tt[:], AF.Sigmoid, scale=CAP)
                # c gate: tanh(.)
                ps_c = gate_psum(wqc, sc)
                thc = ptmp.tile([128, 512], f32, tag="thc")
                nc.scalar.activation(thc[:], ps_c[:], AF.Tanh)
                u = ptmp.tile([128, 512], f32, tag="u")
                nc.vector.tensor_mul(u[:], sgi[:], thc[:])
                # f gate
                ps_f = gate_psum(wqf, sc)
                tt2 = ptmp.tile([128, 512], f32, tag="tt")
                nc.scalar.activation(tt2[:], ps_f[:], AF.Tanh, scale=1.0 / CAP)
                sgf = ptmp.tile([128, 512], f32, tag="sgf")
                nc.scalar.activation(sgf[:], tt2[:], AF.Sigmoid, scale=CAP)
                # recurrence h = f*h + u along the free (time) axis
                hc = ptmp.tile([128, 512], f32, tag="h")
                init = h0s[:, pt:pt + 1] if sc == 0 else h_prev[:, 511:512]
                nc.vector.tensor_tensor_scan(hc[:], sgf[:], u[:], init,
                                             ALU.mult, ALU.add)
                h_prev = hc
                th = ptmp.tile([128, 512], f32, tag="th")
                nc.scalar.activation(th[:], hc[:], AF.Tanh)
                # o gate
                ps_o = gate_psum(wqo, sc)
                tt3 = ptmp.tile([128, 512], f32, tag="tt")
                nc.scalar.activation(tt3[:], ps_o[:], AF.Tanh, scale=1.0 / CAP)
                sgo = ptmp.tile([128, 512], f32, tag="sgo")
                nc.scalar.activation(sgo[:], tt3[:], AF.Sigmoid, scale=CAP)
                nc.vector.tensor_mul(ot[:, pt, ssl], sgo[:], th[:])
                if sc == NSC - 1:
                    nc.vector.tensor_copy(hfs[:, pt:pt + 1], hc[:, 511:512])
        nc.gpsimd.dma_start(hf.rearrange("(t p) -> p t", p=128), hfs[:])

        # ---- phase 2: output projection + residual ----
        # Loads (wot, residual) ride the scalar-engine HWDGE FIFO; stores
        # stay on sync's — two independent rings, so stores don't block
        # the loads the tensor engine is waiting on.
        wo_r = wo.rearrange("(t p) d -> p t d", p=128)
        for dt in range(DT):
            d0 = dt * 512
            dn = min(512, DL - d0)
            dsl = slice(d0, d0 + dn)
            wot = pw2.tile([128, npt, dn], bf16, tag="wot")
            nc.scalar.dma_start(wot[:], wo_r[:, :, dsl])
            for st in range(ST):
                stsl = slice(st * 128, (st + 1) * 128)
                ps = ppsum2.tile([128, dn], f32, tag="ps2")
                for kt in range(npt):
                    nc.tensor.matmul(ps[:], ot[:, kt, stsl], wot[:, kt, :],
                                     start=(kt == 0), stop=(kt == npt - 1))
                res = pio.tile([128, dn], f32, tag="res")
                nc.scalar.dma_start(res[:], xr[stsl, dsl])
                oc = pio.tile([128, dn], f32, tag="oc")
                nc.vector.tensor_add(oc[:], ps[:], res[:])
                nc.sync.dma_start(out[stsl, dsl], oc[:])

    nc.compile()
    return nc


def _shard_inputs(x, hidden_state, Wi, Wf, Wo, Wc, Wout, ln_weight):
    import ml_dtypes
    bf = ml_dtypes.bfloat16
    KD = D // 128

    # fold ln_weight into the gate weights; builder gate order is i,c,f,o
    gates = [Wi * ln_weight[None, :], Wc * ln_weight[None, :],
             Wf * ln_weight[None, :], Wo * ln_weight[None, :]]

    per_j = []
    for j in range(2):
        lo = 0 if j == 0 else P0
        hi = P0 if j == 0 else P
        n = hi - lo
        wgb = np.zeros((NPT, 128, 4, KD, 128), dtype=bf)
        for gi, Wg in enumerate(gates):
            pad = np.zeros((PH, D), dtype=np.float32)
            pad[:n] = Wg[lo:hi]
            r = pad.reshape(NPT, 128, KD, 128)          # [pt, m, kd, dp]
            wgb[:, :, gi, :, :] = r.transpose(0, 3, 2, 1).astype(bf)
        wob = np.zeros((PH, D), dtype=bf)
        wob[:n] = Wout[:, lo:hi].T.astype(bf)
        per_j.append((wgb, wob, lo, hi, n))

    zeros_res = np.zeros((S, D), dtype=np.float32)
    in_maps = []
    for b in range(B):
        xt_b = x[b].T.astype(bf)
        for j in range(2):
            wgb, wob, lo, hi, n = per_j[j]
            h0b = np.zeros(PH, dtype=np.float32)
            h0b[:n] = hidden_state[b, lo:hi]
            in_maps.append({
                "xt": xt_b,
                "xr": x[b] if j == 0 else zeros_res,
                "wg": wgb,
                "wo": wob,
                "h0": h0b,
            })
    return in_maps


def kernel(x, hidden_state, Wi, Wf, Wo, Wc, Wout, ln_weight, _trace=False):
    from concourse.bass_utils import run_bass_kernel_spmd

    x = np.asarray(x, dtype=np.float32)
    hidden_state = np.asarray(hidden_state, dtype=np.float32)
    Wi = np.asarray(Wi, dtype=np.float32)
    Wf = np.asarray(Wf, dtype=np.float32)
    Wo = np.asarray(Wo, dtype=np.float32)
    Wc = np.asarray(Wc, dtype=np.float32)
    Wout = np.asarray(Wout, dtype=np.float32)
    ln_weight = np.asarray(ln_weight, dtype=np.float32)

    global _NC
    if _NC is None:
        _NC = build()

    in_maps = _shard_inputs(x, hidden_state, Wi, Wf, Wo, Wc, Wout, ln_weight)
    res = run_bass_kernel_spmd(_NC, in_maps, list(range(N_CORES)),
                               trace=_trace)

    out = np.empty((B, S, D), dtype=np.float32)
    h_final = np.empty((B, P), dtype=np.float32)
    for b in range(B):
        r0, r1 = res.results[2 * b], res.results[2 * b + 1]
        out[b] = r0["out"] + r1["out"]
        h_final[b, :P0] = r0["hf"][:P0]
        h_final[b, P0:] = r1["hf"][:P - P0]
    if _trace:
        kernel.last_exec_time_ns = res.exec_time_ns
    return out, h_final
